# revision 3
# baseline (speedup 1.0000x reference)
"""Trainium2 Bass kernel for GNN message passing:
    out[i] = sum_{e: dst[e]==i} x[src[e]]     (x: [N, 64] f32, edge_index: [2, E] int)

Strategy (graph-partitioned node sharding, 8 cores):
  * Host sorts edges by destination and shards the destination-node space
    across the 8 cores (N/8 nodes per core, x replicated). Each core's
    128-node destination tiles are permuted so heavy tiles align across
    cores (minimizes union padding; host un-permutes rows at the end).
  * x is repacked as [N, 128] bf16 rows: [bf16(x) | bf16(x - bf16(x))]
    (hi|lo split): one 256 B-row gather fetches both halves, one bf16
    matmul per chunk processes both, and they are summed at evacuation —
    ~1e-5 relative accuracy at bf16 PE speed.
  * Edges are grouped per (supertile of 8 dst tiles, source block of 25000
    rows — int16-safe for dma_gather) into contiguous runs, padded only to
    the 128-edge chunk size. Chunks may straddle destination-tile
    boundaries; every (chunk, tile) pair present in ANY core gets a matmul
    slot, and per-core local-dst streams mask foreign edges with -1.
  * Gather descriptor generation is the bottleneck (SWDGE runs on one Q7
    core pair); calls round-robin across 4 SWDGE queues so all four core
    pairs emit descriptors in parallel.
  * Per core, per chunk: dma_gather (GPSIMD) fetches packed rows; VectorE
    builds the chunk's one-hot with a single tensor_scalar is_equal
    against an fp16 iota covering the chunk's contiguous span of dst
    tiles (per-partition f32 scalar = the edge's span-local dst, -1 for
    foreign/padding); TensorE accumulates psum[tile] += onehot.T @ msgs
    (one PSUM bank per live tile); ScalarE+VectorE merge hi+lo into SBUF
    staging at supertile end.
  * Each core stores its padded [N/8, 64] f32 slice with one DMA; the host
    un-permutes tile rows and concatenates. No collectives.
"""

import numpy as np
import ml_dtypes

import concourse.bacc as bacc
import concourse.bass as bass
import concourse.mybir as mybir
import concourse.tile as tile
from concourse.bass_utils import run_bass_kernel_spmd

P = 128
F32 = mybir.dt.float32
F16 = mybir.dt.float16
BF16 = mybir.dt.bfloat16
I16 = mybir.dt.int16
I32 = mybir.dt.int32
BF = ml_dtypes.bfloat16

# Full-problem constants (hardcoded per harness contract).
N_NODES = 100000
DIM = 64
N_CORES = 8
SRC_BLOCK = 25000        # int16-safe source block
CHUNKS_PER_CALL = 48     # max chunks per dma_gather call; split packets
SUPERTILE = 8            # dst tiles per supertile (<= 8 PSUM banks live)
N_QUEUES = 4             # SWDGE queues (each runs on its own Q7 core pair)
SINGLE_PACKET = False    # single_packet caps at 64 ring descriptors


def _prep(edge_index, n_nodes, n_cores, block, w, stile=SUPERTILE):
    npc = n_nodes // n_cores
    tiles = -(-npc // P)
    nblocks = -(-n_nodes // block)
    n_super = -(-tiles // stile)

    dst = np.asarray(edge_index[0]).astype(np.int64)
    src = np.asarray(edge_index[1]).astype(np.int64)

    k_of = dst // npc
    t_of = (dst - k_of * npc) // P
    b_of = src // block
    seg = (k_of * tiles + t_of) * nblocks + b_of
    order = np.argsort(seg, kind="stable")
    dst_s = dst[order]
    src_s = src[order]
    seg_s = seg[order]

    counts0 = np.bincount(
        seg_s, minlength=n_cores * tiles * nblocks
    ).reshape(n_cores, tiles, nblocks)
    # global start offset of each (core, true tile, block) bucket
    all_starts = np.concatenate(
        [[0], np.cumsum(counts0.ravel())]
    )

    # tile -> slot permutation per core (align heavy tiles across cores)
    perm = np.argsort(-counts0.sum(axis=2), axis=1, kind="stable")  # [cores, tiles]
    counts = np.take_along_axis(counts0, perm[:, :, None], axis=1)  # [cores,slot,b]

    # ---- runs: (supertile, block) -> concatenated slot buckets, pad to 128
    chunk_block = []
    chunk_super = []
    run_meta = []  # (s, b, chunk0, nch, ends_k [cores, nts], ts)
    for s in range(n_super):
        ts = list(range(s * stile, min((s + 1) * stile, tiles)))
        for b in range(nblocks):
            c_kt = counts[:, ts, b]                      # [cores, nts]
            run_max = int(c_kt.sum(axis=1).max())
            if run_max == 0:
                continue
            nch = -(-run_max // P)
            chunk0 = len(chunk_block)
            chunk_block += [b] * nch
            chunk_super += [s] * nch
            run_meta.append((s, b, chunk0, nch, np.cumsum(c_kt, axis=1), ts))
    ch = len(chunk_block)
    chunk_block = np.array(chunk_block)
    chunk_super = np.array(chunk_super)

    # ---- matmul slots: union over cores of tiles present in each chunk
    mm_chunk = []
    mm_tile = []
    for s, b, chunk0, nch, ends_k, ts in run_meta:
        starts_k = ends_k - counts[:, ts, b]
        for ci_local in range(nch):
            a0, a1 = ci_local * P, (ci_local + 1) * P
            present = ((starts_k < a1) & (ends_k > a0)).any(axis=0)
            for j in np.nonzero(present)[0]:
                mm_chunk.append(chunk0 + ci_local)
                mm_tile.append(ts[j])
    nslots = len(mm_chunk)
    mm_chunk = np.array(mm_chunk)
    mm_tile = np.array(mm_tile)

    mm_first = np.zeros(nslots, dtype=bool)
    mm_last = np.zeros(nslots, dtype=bool)
    seen = set()
    for i in range(nslots):
        t = int(mm_tile[i])
        if t not in seen:
            seen.add(t)
            mm_first[i] = True
    seen = set()
    for i in range(nslots - 1, -1, -1):
        t = int(mm_tile[i])
        if t not in seen:
            seen.add(t)
            mm_last[i] = True
    tile_has = np.zeros(tiles, dtype=bool)
    if nslots:
        tile_has[np.unique(mm_tile)] = True

    # ---- per-chunk base tile (min tile among the chunk's slots) and span
    chunk_t0 = np.full(ch, -1, np.int64)
    chunk_t1 = np.full(ch, -1, np.int64)
    for i in range(nslots):
        c = int(mm_chunk[i])
        t = int(mm_tile[i])
        if chunk_t0[c] < 0 or t < chunk_t0[c]:
            chunk_t0[c] = t
        if t > chunk_t1[c]:
            chunk_t1[c] = t
    chunk_t0[chunk_t0 < 0] = 0
    chunk_t1[chunk_t1 < 0] = 0
    chunk_span = chunk_t1 - chunk_t0 + 1
    max_span = int(chunk_span.max()) if ch else 1

    # ---- calls: split each run into <= w chunk pieces (same block)
    calls = []  # (block, c0, csize, slot0, nslots_call)
    for s, b, chunk0, nch, ends_k, ts in run_meta:
        start = chunk0
        while start < chunk0 + nch:
            csize = min(w, chunk0 + nch - start)
            s0 = int(np.searchsorted(mm_chunk, start))
            s1 = int(np.searchsorted(mm_chunk, start + csize))
            calls.append((b, start, csize, s0, s1 - s0))
            start += csize

    # ---- per-core streams
    idx_flat = np.zeros((n_cores, ch * P), np.int16)
    # per-chunk span-local dst (f32; -1 = foreign core or padding)
    ldst_chunk = np.full((n_cores, ch, P), -1.0, np.float32)
    for k in range(n_cores):
        for s, b, chunk0, nch, ends_k, ts in run_meta:
            pieces_src = []
            pieces_ldst = []
            pieces_slot = []
            for j, t in enumerate(ts):
                cnt = int(counts[k, t, b])
                if cnt == 0:
                    continue
                tt = int(perm[k, t])
                g0 = int(all_starts[(k * tiles + tt) * nblocks + b])
                pieces_src.append(src_s[g0 : g0 + cnt] - b * block)
                pieces_ldst.append(dst_s[g0 : g0 + cnt] - (k * npc + tt * P))
                pieces_slot.append(np.full(cnt, t, np.int64))
            if not pieces_src:
                continue
            esrc = np.concatenate(pieces_src).astype(np.int16)
            eldst = np.concatenate(pieces_ldst)
            eslot = np.concatenate(pieces_slot)
            n_e = esrc.shape[0]
            base = chunk0 * P
            idx_flat[k, base : base + n_e] = esrc
            for ci_local in range(nch):
                c = chunk0 + ci_local
                a0 = ci_local * P
                a1 = min(a0 + P, n_e)
                if a0 >= n_e:
                    continue
                # span-local dst value within this chunk's tile window
                vals = (
                    (eslot[a0:a1] - chunk_t0[c]) * P + eldst[a0:a1]
                ).astype(np.float32)
                ldst_chunk[k, c, : a1 - a0] = vals

    idx_all = np.ascontiguousarray(
        np.tile(idx_flat.reshape(n_cores, ch * 8, 16).transpose(0, 2, 1), (1, 8, 1))
    )
    ldst_all = np.ascontiguousarray(ldst_chunk.transpose(0, 2, 1))  # [cores,P,ch]

    return dict(
        npc=npc,
        tiles=tiles,
        nblocks=nblocks,
        n_super=n_super,
        stile=stile,
        ch=ch,
        nslots=nslots,
        calls=calls,
        chunk_super=chunk_super,
        chunk_t0=chunk_t0,
        max_span=max_span,
        mm_chunk=mm_chunk,
        mm_tile=mm_tile,
        mm_first=mm_first,
        mm_last=mm_last,
        tile_has_chunks=tile_has,
        idx=idx_all,
        ldst=ldst_all,
        perm=perm,
    )


def _pack_x(x):
    """[N, D] f32 -> [N, 2D] bf16 rows: [hi | lo]."""
    x = np.asarray(x, np.float32)
    hi = x.astype(BF)
    lo = (x - hi.astype(np.float32)).astype(BF)
    return np.ascontiguousarray(np.concatenate([hi, lo], axis=1))


def _build(n_nodes, dim, block, w, sched):
    tiles = sched["tiles"]
    stile = sched["stile"]
    n_super = sched["n_super"]
    ch = sched["ch"]
    calls = sched["calls"]
    chunk_super = sched["chunk_super"]
    chunk_t0 = sched["chunk_t0"]
    max_span = sched["max_span"]
    mm_chunk = sched["mm_chunk"]
    mm_tile = sched["mm_tile"]
    mm_first = sched["mm_first"]
    mm_last = sched["mm_last"]
    tile_has = sched["tile_has_chunks"]
    out_pad = tiles * P
    elem = 2 * dim  # packed bf16 row length

    nc = bacc.Bacc(
        "TRN2", target_bir_lowering=False, debug=False, num_swdge_queues=N_QUEUES
    )
    x_t = nc.dram_tensor("xpack", [n_nodes, elem], BF16, kind="ExternalInput")
    idx_t = nc.dram_tensor("idx", [P, ch * 8], I16, kind="ExternalInput")
    ldst_t = nc.dram_tensor("ldst", [P, ch], F32, kind="ExternalInput")
    out_t = nc.dram_tensor("out", [out_pad, dim], F32, kind="ExternalOutput")

    with tile.TileContext(nc) as tc:
        with (
            tc.tile_pool(name="const", bufs=1) as const_pool,
            tc.tile_pool(name="meta", bufs=8) as meta_pool,
            tc.tile_pool(name="gather", bufs=6) as gather_pool,
            tc.tile_pool(name="oh", bufs=8) as oh_pool,
            tc.tile_pool(name="stage", bufs=1) as stage_pool,
            tc.tile_pool(name="psum", bufs=8, space="PSUM") as psum_pool,
        ):
            iota_i = const_pool.tile([P, max_span * P], I32)
            nc.gpsimd.iota(
                iota_i[:], pattern=[[1, max_span * P]], base=0, channel_multiplier=0
            )
            iota_h = const_pool.tile([P, max_span * P], F16)
            nc.vector.tensor_copy(iota_h[:], iota_i[:])

            stage = stage_pool.tile([P, tiles * dim], F32)
            nc.vector.memset(stage[:], 0.0)

            call_idx = 0
            psums = {}
            for s in range(n_super):
                ts = list(range(s * stile, min((s + 1) * stile, tiles)))
                while call_idx < len(calls):
                    b, c0, csize, s0, nsc = calls[call_idx]
                    if int(chunk_super[c0]) != s:
                        break
                    queue = call_idx % N_QUEUES
                    call_idx += 1
                    idx_tile = meta_pool.tile([P, w * 8], I16, tag="idx")
                    nc.sync.dma_start(
                        idx_tile[:, : csize * 8],
                        idx_t[:, c0 * 8 : (c0 + csize) * 8],
                    )
                    ldst_tile = meta_pool.tile([P, w], F32, tag="ldst")
                    nc.sync.dma_start(
                        ldst_tile[:, :csize], ldst_t[:, c0 : c0 + csize]
                    )
                    msgs = gather_pool.tile([P, w, elem], BF16)
                    nc.gpsimd.dma_gather(
                        out_ap=msgs[:, :csize, :],
                        in_ap=x_t[b * block : min((b + 1) * block, n_nodes), :],
                        idxs_ap=idx_tile[:, : csize * 8],
                        num_idxs=csize * P,
                        num_idxs_reg=csize * P,
                        elem_size=elem,
                        single_packet=SINGLE_PACKET,
                        queue_num=queue,
                    )
                    si = s0
                    while si < s0 + nsc:
                        c = int(mm_chunk[si])
                        sj = si
                        while sj < s0 + nsc and int(mm_chunk[sj]) == c:
                            sj += 1
                        t0 = int(chunk_t0[c])
                        span = int(mm_tile[sj - 1]) - t0 + 1
                        cin = c - c0
                        onehot = oh_pool.tile(
                            [P, max_span * P], BF16, name="oh", tag="oh"
                        )
                        nc.vector.tensor_scalar(
                            out=onehot[:, : span * P],
                            in0=iota_h[:, : span * P],
                            scalar1=ldst_tile[:, cin : cin + 1],
                            scalar2=None,
                            op0=mybir.AluOpType.is_equal,
                        )
                        for sk in range(si, sj):
                            t = int(mm_tile[sk])
                            if mm_first[sk]:
                                psums[t] = psum_pool.tile(
                                    [P, elem], F32, tag="ps", name=f"ps{t}"
                                )
                            nc.tensor.matmul(
                                psums[t][:, :],
                                lhsT=onehot[:, (t - t0) * P : (t - t0 + 1) * P],
                                rhs=msgs[:, cin, :],
                                start=bool(mm_first[sk]),
                                stop=bool(mm_last[sk]),
                            )
                        si = sj
                # evacuate: stage[:, t*dim:+dim] = psum_hi + psum_lo
                for t in ts:
                    if not tile_has[t]:
                        continue
                    ps = psums.pop(t)
                    nc.scalar.copy(stage[:, t * dim : (t + 1) * dim], ps[:, :dim])
                    nc.vector.tensor_tensor(
                        out=stage[:, t * dim : (t + 1) * dim],
                        in0=stage[:, t * dim : (t + 1) * dim],
                        in1=ps[:, dim:],
                        op=mybir.AluOpType.add,
                    )

            out_view = out_t[:, :].rearrange("(t p) d -> p t d", p=P)
            nc.sync.dma_start(out_view, stage[:])

    nc.compile()
    return nc


def _run(x, edge_index, n_nodes, dim, n_cores, block, w, **run_kwargs):
    sched = _prep(edge_index, n_nodes, n_cores, block, w)
    xp = _pack_x(x)
    nc = _build(n_nodes, dim, block, w, sched)
    in_maps = [
        {"xpack": xp, "idx": sched["idx"][k], "ldst": sched["ldst"][k]}
        for k in range(n_cores)
    ]
    res = run_bass_kernel_spmd(
        nc, in_maps, core_ids=list(range(n_cores)), **run_kwargs
    )
    npc = sched["npc"]
    tiles = sched["tiles"]
    perm = sched["perm"]
    parts = []
    for k in range(n_cores):
        r = res.results[k]["out"].reshape(tiles, P, -1)
        inv = np.empty(tiles, np.int64)
        inv[perm[k]] = np.arange(tiles)
        parts.append(r[inv].reshape(tiles * P, -1)[:npc])
    out = np.concatenate(parts, axis=0)
    return out, res


def kernel(x, edge_index):
    out, _ = _run(
        x, edge_index, N_NODES, DIM, N_CORES, SRC_BLOCK, CHUNKS_PER_CALL
    )
    return out


# revision 6
# speedup vs baseline: 2.5653x; 2.5653x over previous
"""Trainium2 Bass kernel for GNN message passing:
    out[i] = sum_{e: dst[e]==i} x[src[e]]     (x: [N, 64] f32, edge_index: [2, E] int)

Strategy (graph-partitioned node sharding, 8 cores):
  * Host sorts edges by destination and shards the destination-node space
    across the 8 cores (N/8 nodes per core, x replicated). Each core's
    128-node destination tiles are permuted so heavy tiles align across
    cores (minimizes union padding; host un-permutes rows at the end).
  * x is repacked as [N, 128] bf16 rows: [bf16(x) | bf16(x - bf16(x))]
    (hi|lo split): one 256 B-row gather fetches both halves, one bf16
    matmul per chunk processes both, and they are summed at evacuation —
    ~1e-5 relative accuracy at bf16 PE speed.
  * Edges are grouped per (supertile of 8 dst tiles, source block of 25000
    rows — int16-safe for dma_gather) into contiguous runs, padded only to
    the 128-edge chunk size. Chunks may straddle destination-tile
    boundaries; every (chunk, tile) pair present in ANY core gets a matmul
    slot, and per-core local-dst streams mask foreign edges with -1.
  * Gather descriptor emission (SWDGE, one Q7 core pair per queue) is the
    bottleneck; calls round-robin across 4 SWDGE queues so all four core
    pairs emit concurrently.  Every call passes the SAME num_idxs register
    (hoisted, written once) and a full-width idx stream padded with
    trailing -1s (the ucode trims them), so no per-call register MOVE
    creates a WAR hazard that would serialize dispatch.
  * Per core, per chunk: dma_gather (GPSIMD) fetches packed rows; VectorE
    builds [128,128] bf16 one-hots (fused 4 slots per tensor_tensor
    is_equal against a replicated iota); TensorE accumulates
    psum[tile] += onehot.T @ msgs (one PSUM bank per live tile);
    ScalarE+VectorE merge hi+lo into SBUF staging at supertile end.
  * Each core stores its padded [N/8, 64] f32 slice with one DMA; the host
    un-permutes tile rows and concatenates. No collectives.
"""

import numpy as np
import ml_dtypes

import concourse.bacc as bacc
import concourse.bass as bass
import concourse.mybir as mybir
import concourse.tile as tile
from concourse.bass_utils import run_bass_kernel_spmd

P = 128
F32 = mybir.dt.float32
BF16 = mybir.dt.bfloat16
I16 = mybir.dt.int16
I32 = mybir.dt.int32
BF = ml_dtypes.bfloat16

# Full-problem constants (hardcoded per harness contract).
N_NODES = 100000
DIM = 64
N_CORES = 8
SRC_BLOCK = 25000        # int16-safe source block
CHUNKS_PER_CALL = 12     # max chunks per dma_gather call; split packets
SUPERTILE = 8            # dst tiles per supertile (<= 8 PSUM banks live)
N_QUEUES = 4             # SWDGE queues (each runs on its own Q7 core pair)
SINGLE_PACKET = False    # single_packet caps at 64 ring descriptors


def _prep(edge_index, n_nodes, n_cores, block, w, stile=SUPERTILE):
    npc = n_nodes // n_cores
    tiles = -(-npc // P)
    nblocks = -(-n_nodes // block)
    n_super = -(-tiles // stile)

    dst = np.asarray(edge_index[0]).astype(np.int64)
    src = np.asarray(edge_index[1]).astype(np.int64)

    k_of = dst // npc
    t_of = (dst - k_of * npc) // P
    b_of = src // block
    seg = (k_of * tiles + t_of) * nblocks + b_of
    order = np.argsort(seg, kind="stable")
    dst_s = dst[order]
    src_s = src[order]
    seg_s = seg[order]

    counts0 = np.bincount(
        seg_s, minlength=n_cores * tiles * nblocks
    ).reshape(n_cores, tiles, nblocks)
    # global start offset of each (core, true tile, block) bucket
    all_starts = np.concatenate(
        [[0], np.cumsum(counts0.ravel())]
    )

    # tile -> slot permutation per core (align heavy tiles across cores)
    perm = np.argsort(-counts0.sum(axis=2), axis=1, kind="stable")  # [cores, tiles]
    counts = np.take_along_axis(counts0, perm[:, :, None], axis=1)  # [cores,slot,b]

    # ---- runs: (supertile, block) -> concatenated slot buckets, pad to 128
    chunk_block = []
    chunk_super = []
    run_meta = []  # (s, b, chunk0, nch, ends_k [cores, nts], ts)
    for s in range(n_super):
        ts = list(range(s * stile, min((s + 1) * stile, tiles)))
        for b in range(nblocks):
            c_kt = counts[:, ts, b]                      # [cores, nts]
            run_max = int(c_kt.sum(axis=1).max())
            if run_max == 0:
                continue
            nch = -(-run_max // P)
            chunk0 = len(chunk_block)
            chunk_block += [b] * nch
            chunk_super += [s] * nch
            run_meta.append((s, b, chunk0, nch, np.cumsum(c_kt, axis=1), ts))
    ch = len(chunk_block)
    chunk_block = np.array(chunk_block)
    chunk_super = np.array(chunk_super)

    # ---- matmul slots: union over cores of tiles present in each chunk
    mm_chunk = []
    mm_tile = []
    for s, b, chunk0, nch, ends_k, ts in run_meta:
        starts_k = ends_k - counts[:, ts, b]
        for ci_local in range(nch):
            a0, a1 = ci_local * P, (ci_local + 1) * P
            present = ((starts_k < a1) & (ends_k > a0)).any(axis=0)
            for j in np.nonzero(present)[0]:
                mm_chunk.append(chunk0 + ci_local)
                mm_tile.append(ts[j])
    nslots = len(mm_chunk)
    mm_chunk = np.array(mm_chunk)
    mm_tile = np.array(mm_tile)

    mm_first = np.zeros(nslots, dtype=bool)
    mm_last = np.zeros(nslots, dtype=bool)
    seen = set()
    for i in range(nslots):
        t = int(mm_tile[i])
        if t not in seen:
            seen.add(t)
            mm_first[i] = True
    seen = set()
    for i in range(nslots - 1, -1, -1):
        t = int(mm_tile[i])
        if t not in seen:
            seen.add(t)
            mm_last[i] = True
    tile_has = np.zeros(tiles, dtype=bool)
    if nslots:
        tile_has[np.unique(mm_tile)] = True

    # ---- calls: split each run into <= w chunk pieces (same block)
    calls = []  # (block, c0, csize, slot0, nslots_call)
    for s, b, chunk0, nch, ends_k, ts in run_meta:
        start = chunk0
        while start < chunk0 + nch:
            csize = min(w, chunk0 + nch - start)
            s0 = int(np.searchsorted(mm_chunk, start))
            s1 = int(np.searchsorted(mm_chunk, start + csize))
            calls.append((b, start, csize, s0, s1 - s0))
            start += csize
    max_slots_call = max(c[4] for c in calls)

    # ---- per-core streams
    idx_flat = np.zeros((n_cores, ch * P), np.int16)
    ldst_slots = np.full((n_cores, nslots, P), -1.0, BF)
    for k in range(n_cores):
        for s, b, chunk0, nch, ends_k, ts in run_meta:
            pieces_src = []
            pieces_ldst = []
            pieces_slot = []
            for j, t in enumerate(ts):
                cnt = int(counts[k, t, b])
                if cnt == 0:
                    continue
                tt = int(perm[k, t])
                g0 = int(all_starts[(k * tiles + tt) * nblocks + b])
                pieces_src.append(src_s[g0 : g0 + cnt] - b * block)
                pieces_ldst.append(dst_s[g0 : g0 + cnt] - (k * npc + tt * P))
                pieces_slot.append(np.full(cnt, t, np.int64))
            if not pieces_src:
                continue
            esrc = np.concatenate(pieces_src).astype(np.int16)
            eldst = np.concatenate(pieces_ldst)
            eslot = np.concatenate(pieces_slot)
            n_e = esrc.shape[0]
            base = chunk0 * P
            idx_flat[k, base : base + n_e] = esrc
            s0 = int(np.searchsorted(mm_chunk, chunk0))
            s1 = int(np.searchsorted(mm_chunk, chunk0 + nch))
            for i in range(s0, s1):
                ci_local = int(mm_chunk[i]) - chunk0
                t = int(mm_tile[i])
                a0 = ci_local * P
                a1 = min(a0 + P, n_e)
                if a0 >= n_e:
                    continue
                m = eslot[a0:a1] == t
                if not m.any():
                    continue
                col = ldst_slots[k, i]
                col[: a1 - a0][m] = eldst[a0:a1][m].astype(BF)

    # wrap indices into 16 partitions, replicate across the 8 core groups
    idx_wrapped = np.tile(
        idx_flat.reshape(n_cores, ch * 8, 16).transpose(0, 2, 1), (1, 8, 1)
    )  # [cores, 128, ch*8]

    # per-call full-width idx stream: call j occupies cols [j*w*8,(j+1)*w*8);
    # tail cols beyond csize*8 are 0 (a valid row — gathered then discarded;
    # trailing -1 trim corrupts ring bookkeeping when queues are reused), so
    # every call passes the same num_idxs = w*128 and shares one register.
    ncalls = len(calls)
    idx_calls = np.zeros((n_cores, P, ncalls * w * 8), np.int16)
    for j, (b, c0, csize, s0, nsc) in enumerate(calls):
        idx_calls[:, :, j * w * 8 : j * w * 8 + csize * 8] = idx_wrapped[
            :, :, c0 * 8 : (c0 + csize) * 8
        ]
    idx_all = np.ascontiguousarray(idx_calls)
    ldst_all = np.ascontiguousarray(ldst_slots.transpose(0, 2, 1))  # [cores,P,nslots]

    return dict(
        npc=npc,
        tiles=tiles,
        nblocks=nblocks,
        n_super=n_super,
        stile=stile,
        ch=ch,
        nslots=nslots,
        calls=calls,
        max_slots_call=max_slots_call,
        chunk_super=chunk_super,
        mm_chunk=mm_chunk,
        mm_tile=mm_tile,
        mm_first=mm_first,
        mm_last=mm_last,
        tile_has_chunks=tile_has,
        idx=idx_all,
        ldst=ldst_all,
        perm=perm,
    )


def _pack_x(x):
    """[N, D] f32 -> [N, 2D] bf16 rows: [hi | lo]."""
    x = np.asarray(x, np.float32)
    hi = x.astype(BF)
    lo = (x - hi.astype(np.float32)).astype(BF)
    return np.ascontiguousarray(np.concatenate([hi, lo], axis=1))


def _build(n_nodes, dim, block, w, sched):
    tiles = sched["tiles"]
    stile = sched["stile"]
    n_super = sched["n_super"]
    nslots = sched["nslots"]
    calls = sched["calls"]
    msc = sched["max_slots_call"]
    chunk_super = sched["chunk_super"]
    mm_chunk = sched["mm_chunk"]
    mm_tile = sched["mm_tile"]
    mm_first = sched["mm_first"]
    mm_last = sched["mm_last"]
    tile_has = sched["tile_has_chunks"]
    ncalls = len(calls)
    out_pad = tiles * P
    elem = 2 * dim  # packed bf16 row length

    nc = bacc.Bacc(
        "TRN2", target_bir_lowering=False, debug=False, num_swdge_queues=N_QUEUES
    )
    x_t = nc.dram_tensor("xpack", [n_nodes, elem], BF16, kind="ExternalInput")
    idx_t = nc.dram_tensor("idx", [P, ncalls * w * 8], I16, kind="ExternalInput")
    ldst_t = nc.dram_tensor("ldst", [P, nslots], BF16, kind="ExternalInput")
    out_t = nc.dram_tensor("out", [out_pad, dim], F32, kind="ExternalOutput")

    with tile.TileContext(nc) as tc:
        with (
            tc.tile_pool(name="const", bufs=1) as const_pool,
            tc.tile_pool(name="meta", bufs=8) as meta_pool,
            tc.tile_pool(name="gather", bufs=6) as gather_pool,
            tc.tile_pool(name="oh", bufs=8) as oh_pool,
            tc.tile_pool(name="stage", bufs=1) as stage_pool,
            tc.tile_pool(name="psum", bufs=8, space="PSUM") as psum_pool,
        ):
            iota_i = const_pool.tile([P, 4 * P], I32)
            nc.gpsimd.iota(
                iota_i[:], pattern=[[0, 4], [1, P]], base=0, channel_multiplier=0
            )
            iota_b = const_pool.tile([P, 4 * P], BF16)
            nc.vector.tensor_copy(iota_b[:], iota_i[:])

            stage = stage_pool.tile([P, tiles * dim], F32)
            nc.vector.memset(stage[:], 0.0)

            # one shared num_idxs register: written once, read by every
            # gather -> no per-call MOVE / WAR hazard serializing dispatch
            nidx_reg = nc.gpsimd.to_reg(w * P)

            call_idx = 0
            psums = {}
            for s in range(n_super):
                ts = list(range(s * stile, min((s + 1) * stile, tiles)))
                while call_idx < len(calls):
                    b, c0, csize, s0, nsc = calls[call_idx]
                    if int(chunk_super[c0]) != s:
                        break
                    queue = call_idx % N_QUEUES
                    j = call_idx
                    call_idx += 1
                    idx_tile = meta_pool.tile([P, w * 8], I16, tag="idx")
                    nc.sync.dma_start(
                        idx_tile[:],
                        idx_t[:, j * w * 8 : (j + 1) * w * 8],
                    )
                    ldst_tile = meta_pool.tile([P, msc], BF16, tag="ldst")
                    if nsc:
                        nc.sync.dma_start(
                            ldst_tile[:, :nsc], ldst_t[:, s0 : s0 + nsc]
                        )
                    msgs = gather_pool.tile([P, w, elem], BF16)
                    nc.gpsimd.dma_gather(
                        out_ap=msgs[:, :, :],
                        in_ap=x_t[b * block : min((b + 1) * block, n_nodes), :],
                        idxs_ap=idx_tile[:],
                        num_idxs=w * P,
                        num_idxs_reg=nidx_reg,
                        elem_size=elem,
                        single_packet=SINGLE_PACKET,
                        queue_num=queue,
                    )
                    for j0 in range(0, nsc, 4):
                        g = min(4, nsc - j0)
                        onehot = oh_pool.tile([P, 4 * P], BF16, name="oh", tag="oh")
                        lt = ldst_tile[:, j0 : j0 + g]
                        lt_b = bass.AP(lt.tensor, lt.offset, lt.ap + [[0, P]])
                        nc.vector.tensor_tensor(
                            out=onehot[:, : g * P].rearrange(
                                "p (g q) -> p g q", q=P
                            ),
                            in0=iota_b[:, : g * P].rearrange(
                                "p (g q) -> p g q", q=P
                            ),
                            in1=lt_b,
                            op=mybir.AluOpType.is_equal,
                        )
                        for jj in range(g):
                            si = s0 + j0 + jj
                            t = int(mm_tile[si])
                            cin = int(mm_chunk[si]) - c0
                            if mm_first[si]:
                                psums[t] = psum_pool.tile(
                                    [P, elem], F32, tag="ps", name=f"ps{t}"
                                )
                            nc.tensor.matmul(
                                psums[t][:, :],
                                lhsT=onehot[:, jj * P : (jj + 1) * P],
                                rhs=msgs[:, cin, :],
                                start=bool(mm_first[si]),
                                stop=bool(mm_last[si]),
                            )
                # evacuate: stage[:, t*dim:+dim] = psum_hi + psum_lo
                for t in ts:
                    if not tile_has[t]:
                        continue
                    ps = psums.pop(t)
                    nc.scalar.copy(stage[:, t * dim : (t + 1) * dim], ps[:, :dim])
                    nc.vector.tensor_tensor(
                        out=stage[:, t * dim : (t + 1) * dim],
                        in0=stage[:, t * dim : (t + 1) * dim],
                        in1=ps[:, dim:],
                        op=mybir.AluOpType.add,
                    )

            out_view = out_t[:, :].rearrange("(t p) d -> p t d", p=P)
            nc.sync.dma_start(out_view, stage[:])

    nc.compile()
    return nc


def _run(x, edge_index, n_nodes, dim, n_cores, block, w, **run_kwargs):
    sched = _prep(edge_index, n_nodes, n_cores, block, w)
    xp = _pack_x(x)
    nc = _build(n_nodes, dim, block, w, sched)
    in_maps = [
        {"xpack": xp, "idx": sched["idx"][k], "ldst": sched["ldst"][k]}
        for k in range(n_cores)
    ]
    res = run_bass_kernel_spmd(
        nc, in_maps, core_ids=list(range(n_cores)), **run_kwargs
    )
    npc = sched["npc"]
    tiles = sched["tiles"]
    perm = sched["perm"]
    parts = []
    for k in range(n_cores):
        r = res.results[k]["out"].reshape(tiles, P, -1)
        inv = np.empty(tiles, np.int64)
        inv[perm[k]] = np.arange(tiles)
        parts.append(r[inv].reshape(tiles * P, -1)[:npc])
    out = np.concatenate(parts, axis=0)
    return out, res


def kernel(x, edge_index):
    out, _ = _run(
        x, edge_index, N_NODES, DIM, N_CORES, SRC_BLOCK, CHUNKS_PER_CALL
    )
    return out


# revision 7
# speedup vs baseline: 2.6656x; 1.0391x over previous
"""Trainium2 Bass kernel for GNN message passing:
    out[i] = sum_{e: dst[e]==i} x[src[e]]     (x: [N, 64] f32, edge_index: [2, E] int)

Strategy (graph-partitioned node sharding, 8 cores):
  * Host sorts edges by destination and shards the destination-node space
    across the 8 cores (N/8 nodes per core, x replicated). Each core's
    128-node destination tiles are permuted so heavy tiles align across
    cores (minimizes union padding; host un-permutes rows at the end).
  * x is repacked as [N, 128] bf16 rows: [bf16(x) | bf16(x - bf16(x))]
    (hi|lo split): one 256 B-row gather fetches both halves, one bf16
    matmul per chunk processes both, and they are summed at evacuation —
    ~1e-5 relative accuracy at bf16 PE speed.
  * Edges are grouped per (supertile of 8 dst tiles, source block of 25000
    rows — int16-safe for dma_gather) into contiguous runs, padded only to
    the 128-edge chunk size. Chunks may straddle destination-tile
    boundaries; every (chunk, tile) pair present in ANY core gets a matmul
    slot, and per-core local-dst streams mask foreign edges with -1.
  * Gather descriptor emission (SWDGE, one Q7 core pair per queue) is the
    bottleneck; calls round-robin across 4 SWDGE queues so all four core
    pairs emit concurrently.  Every call passes the SAME num_idxs register
    (hoisted, written once) and a full-width idx stream padded with
    trailing -1s (the ucode trims them), so no per-call register MOVE
    creates a WAR hazard that would serialize dispatch.
  * Per core, per chunk: dma_gather (GPSIMD) fetches packed rows; VectorE
    builds [128,128] bf16 one-hots (fused 4 slots per tensor_tensor
    is_equal against a replicated iota); TensorE accumulates
    psum[tile] += onehot.T @ msgs (one PSUM bank per live tile);
    ScalarE+VectorE merge hi+lo into SBUF staging at supertile end.
  * Each core stores its padded [N/8, 64] f32 slice with one DMA; the host
    un-permutes tile rows and concatenates. No collectives.
"""

import numpy as np
import ml_dtypes

import concourse.bacc as bacc
import concourse.bass as bass
import concourse.mybir as mybir
import concourse.tile as tile
from concourse.bass_utils import run_bass_kernel_spmd

P = 128
F32 = mybir.dt.float32
BF16 = mybir.dt.bfloat16
I16 = mybir.dt.int16
I32 = mybir.dt.int32
BF = ml_dtypes.bfloat16

# Full-problem constants (hardcoded per harness contract).
N_NODES = 100000
DIM = 64
N_CORES = 8
SRC_BLOCK = 25000        # int16-safe source block
CHUNKS_PER_CALL = 12     # max chunks per dma_gather call; split packets
SUPERTILE = 8            # dst tiles per supertile (<= 8 PSUM banks live)
N_QUEUES = 4             # SWDGE queues (each runs on its own Q7 core pair)
SINGLE_PACKET = False    # single_packet caps at 64 ring descriptors


def _prep(edge_index, n_nodes, n_cores, block, w, stile=SUPERTILE):
    npc = n_nodes // n_cores
    tiles = -(-npc // P)
    nblocks = -(-n_nodes // block)
    n_super = -(-tiles // stile)

    dst = np.asarray(edge_index[0]).astype(np.int64)
    src = np.asarray(edge_index[1]).astype(np.int64)

    k_of = dst // npc
    t_of = (dst - k_of * npc) // P
    b_of = src // block
    seg = (k_of * tiles + t_of) * nblocks + b_of
    order = np.argsort(seg, kind="stable")
    dst_s = dst[order]
    src_s = src[order]
    seg_s = seg[order]

    counts0 = np.bincount(
        seg_s, minlength=n_cores * tiles * nblocks
    ).reshape(n_cores, tiles, nblocks)
    # global start offset of each (core, true tile, block) bucket
    all_starts = np.concatenate(
        [[0], np.cumsum(counts0.ravel())]
    )

    # tile -> slot permutation per core (align heavy tiles across cores)
    perm = np.argsort(-counts0.sum(axis=2), axis=1, kind="stable")  # [cores, tiles]
    counts = np.take_along_axis(counts0, perm[:, :, None], axis=1)  # [cores,slot,b]

    # ---- runs: (supertile, block) -> concatenated slot buckets, pad to 128
    chunk_block = []
    chunk_super = []
    run_meta = []  # (s, b, chunk0, nch, ends_k [cores, nts], ts)
    for s in range(n_super):
        ts = list(range(s * stile, min((s + 1) * stile, tiles)))
        for b in range(nblocks):
            c_kt = counts[:, ts, b]                      # [cores, nts]
            run_max = int(c_kt.sum(axis=1).max())
            if run_max == 0:
                continue
            nch = -(-run_max // P)
            chunk0 = len(chunk_block)
            chunk_block += [b] * nch
            chunk_super += [s] * nch
            run_meta.append((s, b, chunk0, nch, np.cumsum(c_kt, axis=1), ts))
    ch = len(chunk_block)
    chunk_block = np.array(chunk_block)
    chunk_super = np.array(chunk_super)

    # ---- matmul slots: union over cores of tiles present in each chunk
    mm_chunk = []
    mm_tile = []
    for s, b, chunk0, nch, ends_k, ts in run_meta:
        starts_k = ends_k - counts[:, ts, b]
        for ci_local in range(nch):
            a0, a1 = ci_local * P, (ci_local + 1) * P
            present = ((starts_k < a1) & (ends_k > a0)).any(axis=0)
            for j in np.nonzero(present)[0]:
                mm_chunk.append(chunk0 + ci_local)
                mm_tile.append(ts[j])
    nslots = len(mm_chunk)
    mm_chunk = np.array(mm_chunk)
    mm_tile = np.array(mm_tile)

    mm_first = np.zeros(nslots, dtype=bool)
    mm_last = np.zeros(nslots, dtype=bool)
    seen = set()
    for i in range(nslots):
        t = int(mm_tile[i])
        if t not in seen:
            seen.add(t)
            mm_first[i] = True
    seen = set()
    for i in range(nslots - 1, -1, -1):
        t = int(mm_tile[i])
        if t not in seen:
            seen.add(t)
            mm_last[i] = True
    tile_has = np.zeros(tiles, dtype=bool)
    if nslots:
        tile_has[np.unique(mm_tile)] = True

    # ---- calls: split each run into <= w chunk pieces (same block)
    calls = []  # (block, c0, csize, slot0, nslots_call)
    for s, b, chunk0, nch, ends_k, ts in run_meta:
        start = chunk0
        while start < chunk0 + nch:
            csize = min(w, chunk0 + nch - start)
            s0 = int(np.searchsorted(mm_chunk, start))
            s1 = int(np.searchsorted(mm_chunk, start + csize))
            calls.append((b, start, csize, s0, s1 - s0))
            start += csize
    max_slots_call = max(c[4] for c in calls)

    # ---- per-core streams
    idx_flat = np.zeros((n_cores, ch * P), np.int16)
    ldst_slots = np.full((n_cores, nslots, P), -1.0, BF)
    for k in range(n_cores):
        for s, b, chunk0, nch, ends_k, ts in run_meta:
            pieces_src = []
            pieces_ldst = []
            pieces_slot = []
            for j, t in enumerate(ts):
                cnt = int(counts[k, t, b])
                if cnt == 0:
                    continue
                tt = int(perm[k, t])
                g0 = int(all_starts[(k * tiles + tt) * nblocks + b])
                pieces_src.append(src_s[g0 : g0 + cnt] - b * block)
                pieces_ldst.append(dst_s[g0 : g0 + cnt] - (k * npc + tt * P))
                pieces_slot.append(np.full(cnt, t, np.int64))
            if not pieces_src:
                continue
            esrc = np.concatenate(pieces_src).astype(np.int16)
            eldst = np.concatenate(pieces_ldst)
            eslot = np.concatenate(pieces_slot)
            n_e = esrc.shape[0]
            base = chunk0 * P
            idx_flat[k, base : base + n_e] = esrc
            s0 = int(np.searchsorted(mm_chunk, chunk0))
            s1 = int(np.searchsorted(mm_chunk, chunk0 + nch))
            for i in range(s0, s1):
                ci_local = int(mm_chunk[i]) - chunk0
                t = int(mm_tile[i])
                a0 = ci_local * P
                a1 = min(a0 + P, n_e)
                if a0 >= n_e:
                    continue
                m = eslot[a0:a1] == t
                if not m.any():
                    continue
                col = ldst_slots[k, i]
                col[: a1 - a0][m] = eldst[a0:a1][m].astype(BF)

    # wrap indices into 16 partitions, replicate across the 8 core groups
    idx_wrapped = np.tile(
        idx_flat.reshape(n_cores, ch * 8, 16).transpose(0, 2, 1), (1, 8, 1)
    )  # [cores, 128, ch*8]

    # per-call full-width idx stream: call j occupies cols [j*w*8,(j+1)*w*8);
    # tail cols beyond csize*8 are 0 (a valid row — gathered then discarded;
    # trailing -1 trim corrupts ring bookkeeping when queues are reused), so
    # every call passes the same num_idxs = w*128 and shares one register.
    ncalls = len(calls)
    idx_calls = np.zeros((n_cores, P, ncalls * w * 8), np.int16)
    for j, (b, c0, csize, s0, nsc) in enumerate(calls):
        idx_calls[:, :, j * w * 8 : j * w * 8 + csize * 8] = idx_wrapped[
            :, :, c0 * 8 : (c0 + csize) * 8
        ]
    idx_all = np.ascontiguousarray(idx_calls)
    ldst_all = np.ascontiguousarray(ldst_slots.transpose(0, 2, 1))  # [cores,P,nslots]

    return dict(
        npc=npc,
        tiles=tiles,
        nblocks=nblocks,
        n_super=n_super,
        stile=stile,
        ch=ch,
        nslots=nslots,
        calls=calls,
        max_slots_call=max_slots_call,
        chunk_super=chunk_super,
        mm_chunk=mm_chunk,
        mm_tile=mm_tile,
        mm_first=mm_first,
        mm_last=mm_last,
        tile_has_chunks=tile_has,
        idx=idx_all,
        ldst=ldst_all,
        perm=perm,
    )


def _pack_x(x):
    """[N, D] f32 -> [N, 2D] bf16 rows: [hi | lo]."""
    x = np.asarray(x, np.float32)
    hi = x.astype(BF)
    lo = (x - hi.astype(np.float32)).astype(BF)
    return np.ascontiguousarray(np.concatenate([hi, lo], axis=1))


def _build(n_nodes, dim, block, w, sched):
    tiles = sched["tiles"]
    stile = sched["stile"]
    n_super = sched["n_super"]
    nslots = sched["nslots"]
    calls = sched["calls"]
    msc = sched["max_slots_call"]
    chunk_super = sched["chunk_super"]
    mm_chunk = sched["mm_chunk"]
    mm_tile = sched["mm_tile"]
    mm_first = sched["mm_first"]
    mm_last = sched["mm_last"]
    tile_has = sched["tile_has_chunks"]
    ncalls = len(calls)
    out_pad = tiles * P
    elem = 2 * dim  # packed bf16 row length

    nc = bacc.Bacc(
        "TRN2", target_bir_lowering=False, debug=False, num_swdge_queues=N_QUEUES
    )
    x_t = nc.dram_tensor("xpack", [n_nodes, elem], BF16, kind="ExternalInput")
    idx_t = nc.dram_tensor("idx", [P, ncalls * w * 8], I16, kind="ExternalInput")
    ldst_t = nc.dram_tensor("ldst", [P, nslots], BF16, kind="ExternalInput")
    out_t = nc.dram_tensor("out", [out_pad, dim], F32, kind="ExternalOutput")

    with tile.TileContext(nc) as tc:
        with (
            tc.tile_pool(name="const", bufs=1) as const_pool,
            tc.tile_pool(name="meta", bufs=16) as meta_pool,
            tc.tile_pool(name="gather", bufs=12) as gather_pool,
            tc.tile_pool(name="oh", bufs=12) as oh_pool,
            tc.tile_pool(name="stage", bufs=1) as stage_pool,
            tc.tile_pool(name="psum", bufs=8, space="PSUM") as psum_pool,
        ):
            iota_i = const_pool.tile([P, 4 * P], I32)
            nc.gpsimd.iota(
                iota_i[:], pattern=[[0, 4], [1, P]], base=0, channel_multiplier=0
            )
            iota_b = const_pool.tile([P, 4 * P], BF16)
            nc.vector.tensor_copy(iota_b[:], iota_i[:])

            stage = stage_pool.tile([P, tiles * dim], F32)
            nc.vector.memset(stage[:], 0.0)

            # one shared num_idxs register: written once, read by every
            # gather -> no per-call MOVE / WAR hazard serializing dispatch
            nidx_reg = nc.gpsimd.to_reg(w * P)

            call_idx = 0
            psums = {}
            for s in range(n_super):
                ts = list(range(s * stile, min((s + 1) * stile, tiles)))
                while call_idx < len(calls):
                    b, c0, csize, s0, nsc = calls[call_idx]
                    if int(chunk_super[c0]) != s:
                        break
                    queue = call_idx % N_QUEUES
                    j = call_idx
                    call_idx += 1
                    idx_tile = meta_pool.tile([P, w * 8], I16, tag="idx")
                    nc.sync.dma_start(
                        idx_tile[:],
                        idx_t[:, j * w * 8 : (j + 1) * w * 8],
                    )
                    ldst_tile = meta_pool.tile([P, msc], BF16, tag="ldst")
                    if nsc:
                        nc.sync.dma_start(
                            ldst_tile[:, :nsc], ldst_t[:, s0 : s0 + nsc]
                        )
                    msgs = gather_pool.tile([P, w, elem], BF16)
                    nc.gpsimd.dma_gather(
                        out_ap=msgs[:, :, :],
                        in_ap=x_t[b * block : min((b + 1) * block, n_nodes), :],
                        idxs_ap=idx_tile[:],
                        num_idxs=w * P,
                        num_idxs_reg=nidx_reg,
                        elem_size=elem,
                        single_packet=SINGLE_PACKET,
                        queue_num=queue,
                    )
                    for j0 in range(0, nsc, 4):
                        g = min(4, nsc - j0)
                        onehot = oh_pool.tile([P, 4 * P], BF16, name="oh", tag="oh")
                        lt = ldst_tile[:, j0 : j0 + g]
                        lt_b = bass.AP(lt.tensor, lt.offset, lt.ap + [[0, P]])
                        nc.vector.tensor_tensor(
                            out=onehot[:, : g * P].rearrange(
                                "p (g q) -> p g q", q=P
                            ),
                            in0=iota_b[:, : g * P].rearrange(
                                "p (g q) -> p g q", q=P
                            ),
                            in1=lt_b,
                            op=mybir.AluOpType.is_equal,
                        )
                        for jj in range(g):
                            si = s0 + j0 + jj
                            t = int(mm_tile[si])
                            cin = int(mm_chunk[si]) - c0
                            if mm_first[si]:
                                psums[t] = psum_pool.tile(
                                    [P, elem], F32, tag="ps", name=f"ps{t}"
                                )
                            nc.tensor.matmul(
                                psums[t][:, :],
                                lhsT=onehot[:, jj * P : (jj + 1) * P],
                                rhs=msgs[:, cin, :],
                                start=bool(mm_first[si]),
                                stop=bool(mm_last[si]),
                            )
                # evacuate: stage[:, t*dim:+dim] = psum_hi + psum_lo
                for t in ts:
                    if not tile_has[t]:
                        continue
                    ps = psums.pop(t)
                    nc.scalar.copy(stage[:, t * dim : (t + 1) * dim], ps[:, :dim])
                    nc.vector.tensor_tensor(
                        out=stage[:, t * dim : (t + 1) * dim],
                        in0=stage[:, t * dim : (t + 1) * dim],
                        in1=ps[:, dim:],
                        op=mybir.AluOpType.add,
                    )

            out_view = out_t[:, :].rearrange("(t p) d -> p t d", p=P)
            nc.sync.dma_start(out_view, stage[:])

    nc.compile()
    return nc


def _run(x, edge_index, n_nodes, dim, n_cores, block, w, **run_kwargs):
    sched = _prep(edge_index, n_nodes, n_cores, block, w)
    xp = _pack_x(x)
    nc = _build(n_nodes, dim, block, w, sched)
    in_maps = [
        {"xpack": xp, "idx": sched["idx"][k], "ldst": sched["ldst"][k]}
        for k in range(n_cores)
    ]
    res = run_bass_kernel_spmd(
        nc, in_maps, core_ids=list(range(n_cores)), **run_kwargs
    )
    npc = sched["npc"]
    tiles = sched["tiles"]
    perm = sched["perm"]
    parts = []
    for k in range(n_cores):
        r = res.results[k]["out"].reshape(tiles, P, -1)
        inv = np.empty(tiles, np.int64)
        inv[perm[k]] = np.arange(tiles)
        parts.append(r[inv].reshape(tiles * P, -1)[:npc])
    out = np.concatenate(parts, axis=0)
    return out, res


def kernel(x, edge_index):
    out, _ = _run(
        x, edge_index, N_NODES, DIM, N_CORES, SRC_BLOCK, CHUNKS_PER_CALL
    )
    return out


# revision 11
# speedup vs baseline: 2.6705x; 1.0019x over previous
"""Trainium2 Bass kernel for GNN message passing:
    out[i] = sum_{e: dst[e]==i} x[src[e]]     (x: [N, 64] f32, edge_index: [2, E] int)

Strategy (graph-partitioned node sharding, 8 cores):
  * Host sorts edges by destination and shards the destination-node space
    across the 8 cores (N/8 nodes per core, x replicated). Each core's
    128-node destination tiles are permuted so heavy tiles align across
    cores (minimizes union padding; host un-permutes rows at the end).
  * x is repacked as [N, 128] bf16 rows: [bf16(x) | bf16(x - bf16(x))]
    (hi|lo split): one 256 B-row gather fetches both halves, one bf16
    matmul per chunk processes both, and they are summed at evacuation —
    ~1e-5 relative accuracy at bf16 PE speed.
  * Edges are grouped per (supertile of 8 dst tiles, source block of 25000
    rows — int16-safe for dma_gather) into contiguous runs, padded only to
    the 128-edge chunk size. Chunks may straddle destination-tile
    boundaries; every (chunk, tile) pair present in ANY core gets a matmul
    slot, and per-core local-dst streams mask foreign edges with -1.
  * Gather descriptor emission (SWDGE, one Q7 core pair per queue) is the
    bottleneck; calls round-robin across 4 SWDGE queues so all four core
    pairs emit concurrently.  Every call passes the SAME num_idxs register
    (hoisted, written once) and a full-width idx stream padded with
    trailing -1s (the ucode trims them), so no per-call register MOVE
    creates a WAR hazard that would serialize dispatch.
  * Per core, per chunk: dma_gather (GPSIMD) fetches packed rows; VectorE
    builds [128,128] bf16 one-hots (fused 4 slots per tensor_tensor
    is_equal against a replicated iota); TensorE accumulates
    psum[tile] += onehot.T @ msgs (one PSUM bank per live tile);
    ScalarE+VectorE merge hi+lo into SBUF staging at supertile end.
  * Each core stores its padded [N/8, 64] f32 slice with one DMA; the host
    un-permutes tile rows and concatenates. No collectives.
"""

import numpy as np
import ml_dtypes

import concourse.bacc as bacc
import concourse.bass as bass
import concourse.mybir as mybir
import concourse.tile as tile
from concourse.bass_utils import run_bass_kernel_spmd

P = 128
F32 = mybir.dt.float32
BF16 = mybir.dt.bfloat16
I16 = mybir.dt.int16
I32 = mybir.dt.int32
BF = ml_dtypes.bfloat16

# Full-problem constants (hardcoded per harness contract).
N_NODES = 100000
DIM = 64
N_CORES = 8
SRC_BLOCK = 25000        # int16-safe source block
CHUNKS_PER_CALL = 12     # max chunks per dma_gather call; split packets
SUPERTILE = 8            # dst tiles per supertile (<= 8 PSUM banks live)
N_QUEUES = 4             # SWDGE queues (each runs on its own Q7 core pair)
SINGLE_PACKET = False    # single_packet caps at 64 ring descriptors


def _prep(edge_index, n_nodes, n_cores, block, w, stile=SUPERTILE):
    npc = n_nodes // n_cores
    tiles = -(-npc // P)
    nblocks = -(-n_nodes // block)
    n_super = -(-tiles // stile)

    dst = np.asarray(edge_index[0]).astype(np.int64)
    src = np.asarray(edge_index[1]).astype(np.int64)

    k_of = dst // npc
    t_of = (dst - k_of * npc) // P
    b_of = src // block
    seg = (k_of * tiles + t_of) * nblocks + b_of
    order = np.argsort(seg, kind="stable")
    dst_s = dst[order]
    src_s = src[order]
    seg_s = seg[order]

    counts0 = np.bincount(
        seg_s, minlength=n_cores * tiles * nblocks
    ).reshape(n_cores, tiles, nblocks)
    # global start offset of each (core, true tile, block) bucket
    all_starts = np.concatenate(
        [[0], np.cumsum(counts0.ravel())]
    )

    # tile -> slot permutation per core (align heavy tiles across cores)
    perm = np.argsort(-counts0.sum(axis=2), axis=1, kind="stable")  # [cores, tiles]
    counts = np.take_along_axis(counts0, perm[:, :, None], axis=1)  # [cores,slot,b]

    # ---- runs: (supertile, block) -> concatenated slot buckets, pad to 128
    chunk_block = []
    chunk_super = []
    run_meta = []  # (s, b, chunk0, nch, ends_k [cores, nts], ts)
    for s in range(n_super):
        ts = list(range(s * stile, min((s + 1) * stile, tiles)))
        for b in range(nblocks):
            c_kt = counts[:, ts, b]                      # [cores, nts]
            run_max = int(c_kt.sum(axis=1).max())
            if run_max == 0:
                continue
            nch = -(-run_max // P)
            chunk0 = len(chunk_block)
            chunk_block += [b] * nch
            chunk_super += [s] * nch
            run_meta.append((s, b, chunk0, nch, np.cumsum(c_kt, axis=1), ts))
    ch = len(chunk_block)
    chunk_block = np.array(chunk_block)
    chunk_super = np.array(chunk_super)

    # ---- matmul slots: union over cores of tiles present in each chunk
    mm_chunk = []
    mm_tile = []
    for s, b, chunk0, nch, ends_k, ts in run_meta:
        starts_k = ends_k - counts[:, ts, b]
        for ci_local in range(nch):
            a0, a1 = ci_local * P, (ci_local + 1) * P
            present = ((starts_k < a1) & (ends_k > a0)).any(axis=0)
            for j in np.nonzero(present)[0]:
                mm_chunk.append(chunk0 + ci_local)
                mm_tile.append(ts[j])
    nslots = len(mm_chunk)
    mm_chunk = np.array(mm_chunk)
    mm_tile = np.array(mm_tile)

    mm_first = np.zeros(nslots, dtype=bool)
    mm_last = np.zeros(nslots, dtype=bool)
    seen = set()
    for i in range(nslots):
        t = int(mm_tile[i])
        if t not in seen:
            seen.add(t)
            mm_first[i] = True
    seen = set()
    for i in range(nslots - 1, -1, -1):
        t = int(mm_tile[i])
        if t not in seen:
            seen.add(t)
            mm_last[i] = True
    tile_has = np.zeros(tiles, dtype=bool)
    if nslots:
        tile_has[np.unique(mm_tile)] = True

    # ---- calls: split each run into <= w chunk pieces (same block)
    calls = []  # (block, c0, csize, slot0, nslots_call)
    for s, b, chunk0, nch, ends_k, ts in run_meta:
        start = chunk0
        while start < chunk0 + nch:
            csize = min(w, chunk0 + nch - start)
            s0 = int(np.searchsorted(mm_chunk, start))
            s1 = int(np.searchsorted(mm_chunk, start + csize))
            calls.append((b, start, csize, s0, s1 - s0))
            start += csize
    max_slots_call = max(c[4] for c in calls)

    # ---- per-core streams
    idx_flat = np.zeros((n_cores, ch * P), np.int16)
    ldst_slots = np.full((n_cores, nslots, P), -1.0, BF)
    for k in range(n_cores):
        for s, b, chunk0, nch, ends_k, ts in run_meta:
            pieces_src = []
            pieces_ldst = []
            pieces_slot = []
            for j, t in enumerate(ts):
                cnt = int(counts[k, t, b])
                if cnt == 0:
                    continue
                tt = int(perm[k, t])
                g0 = int(all_starts[(k * tiles + tt) * nblocks + b])
                pieces_src.append(src_s[g0 : g0 + cnt] - b * block)
                pieces_ldst.append(dst_s[g0 : g0 + cnt] - (k * npc + tt * P))
                pieces_slot.append(np.full(cnt, t, np.int64))
            if not pieces_src:
                continue
            esrc = np.concatenate(pieces_src).astype(np.int16)
            eldst = np.concatenate(pieces_ldst)
            eslot = np.concatenate(pieces_slot)
            n_e = esrc.shape[0]
            base = chunk0 * P
            idx_flat[k, base : base + n_e] = esrc
            s0 = int(np.searchsorted(mm_chunk, chunk0))
            s1 = int(np.searchsorted(mm_chunk, chunk0 + nch))
            for i in range(s0, s1):
                ci_local = int(mm_chunk[i]) - chunk0
                t = int(mm_tile[i])
                a0 = ci_local * P
                a1 = min(a0 + P, n_e)
                if a0 >= n_e:
                    continue
                m = eslot[a0:a1] == t
                if not m.any():
                    continue
                col = ldst_slots[k, i]
                col[: a1 - a0][m] = eldst[a0:a1][m].astype(BF)

    # wrap indices into 16 partitions, replicate across the 8 core groups
    idx_wrapped = np.tile(
        idx_flat.reshape(n_cores, ch * 8, 16).transpose(0, 2, 1), (1, 8, 1)
    )  # [cores, 128, ch*8]

    # per-call full-width idx stream: call j occupies cols [j*w*8,(j+1)*w*8);
    # tail cols beyond csize*8 are 0 (a valid row — gathered then discarded;
    # trailing -1 trim corrupts ring bookkeeping when queues are reused), so
    # every call passes the same num_idxs = w*128 and shares one register.
    ncalls = len(calls)
    idx_calls = np.zeros((n_cores, P, ncalls * w * 8), np.int16)
    for j, (b, c0, csize, s0, nsc) in enumerate(calls):
        idx_calls[:, :, j * w * 8 : j * w * 8 + csize * 8] = idx_wrapped[
            :, :, c0 * 8 : (c0 + csize) * 8
        ]
    idx_all = np.ascontiguousarray(idx_calls)

    # ldst packed at a fixed per-call stride (msc slots) so meta DMAs can
    # fetch G calls' worth in one transfer
    ldst_t_slots = ldst_slots.transpose(0, 2, 1)  # [cores, P, nslots]
    ldst_pack = np.full((n_cores, P, ncalls * max_slots_call), -1.0, BF)
    for j, (b, c0, csize, s0, nsc) in enumerate(calls):
        if nsc:
            ldst_pack[:, :, j * max_slots_call : j * max_slots_call + nsc] = (
                ldst_t_slots[:, :, s0 : s0 + nsc]
            )
    ldst_all = np.ascontiguousarray(ldst_pack)

    return dict(
        npc=npc,
        tiles=tiles,
        nblocks=nblocks,
        n_super=n_super,
        stile=stile,
        ch=ch,
        nslots=nslots,
        calls=calls,
        max_slots_call=max_slots_call,
        chunk_super=chunk_super,
        mm_chunk=mm_chunk,
        mm_tile=mm_tile,
        mm_first=mm_first,
        mm_last=mm_last,
        tile_has_chunks=tile_has,
        idx=idx_all,
        ldst=ldst_all,
        perm=perm,
    )


def _pack_x(x):
    """[N, D] f32 -> [N, 2D] bf16 rows: [hi | lo]."""
    x = np.asarray(x, np.float32)
    hi = x.astype(BF)
    lo = (x - hi.astype(np.float32)).astype(BF)
    return np.ascontiguousarray(np.concatenate([hi, lo], axis=1))


def _build(n_nodes, dim, block, w, sched):
    tiles = sched["tiles"]
    stile = sched["stile"]
    n_super = sched["n_super"]
    nslots = sched["nslots"]
    calls = sched["calls"]
    msc = sched["max_slots_call"]
    chunk_super = sched["chunk_super"]
    mm_chunk = sched["mm_chunk"]
    mm_tile = sched["mm_tile"]
    mm_first = sched["mm_first"]
    mm_last = sched["mm_last"]
    tile_has = sched["tile_has_chunks"]
    ncalls = len(calls)
    out_pad = tiles * P
    elem = 2 * dim  # packed bf16 row length

    nc = bacc.Bacc(
        "TRN2", target_bir_lowering=False, debug=False, num_swdge_queues=N_QUEUES
    )
    x_t = nc.dram_tensor("xpack", [n_nodes, elem], BF16, kind="ExternalInput")
    idx_t = nc.dram_tensor("idx", [P, ncalls * w * 8], I16, kind="ExternalInput")
    ldst_t = nc.dram_tensor("ldst", [P, ncalls * msc], BF16, kind="ExternalInput")
    out_t = nc.dram_tensor("out", [out_pad, dim], F32, kind="ExternalOutput")
    G = 8  # calls per meta DMA group

    with tile.TileContext(nc) as tc:
        with (
            tc.tile_pool(name="const", bufs=1) as const_pool,
            tc.tile_pool(name="meta", bufs=16) as meta_pool,
            tc.tile_pool(name="gather", bufs=12) as gather_pool,
            tc.tile_pool(name="oh", bufs=12) as oh_pool,
            tc.tile_pool(name="stage", bufs=1) as stage_pool,
            tc.tile_pool(name="psum", bufs=8, space="PSUM") as psum_pool,
        ):
            iota_i = const_pool.tile([P, 8 * P], I32)
            nc.gpsimd.iota(
                iota_i[:], pattern=[[0, 8], [1, P]], base=0, channel_multiplier=0
            )
            iota_b = const_pool.tile([P, 8 * P], BF16)
            nc.vector.tensor_copy(iota_b[:], iota_i[:])

            stage = stage_pool.tile([P, tiles * dim], F32)
            nc.vector.memset(stage[:], 0.0)

            # one shared num_idxs register: written once, read by every
            # gather -> no per-call MOVE / WAR hazard serializing dispatch
            nidx_reg = nc.gpsimd.to_reg(w * P)

            call_idx = 0
            psums = {}
            for s in range(n_super):
                ts = list(range(s * stile, min((s + 1) * stile, tiles)))
                while call_idx < len(calls):
                    b, c0, csize, s0, nsc = calls[call_idx]
                    if int(chunk_super[c0]) != s:
                        break
                    queue = call_idx % N_QUEUES
                    j = call_idx
                    call_idx += 1
                    if j % G == 0:
                        ng = min(G, ncalls - j)
                        idx_gtile = meta_pool.tile([P, G * w * 8], I16, tag="idx")
                        nc.sync.dma_start(
                            idx_gtile[:, : ng * w * 8],
                            idx_t[:, j * w * 8 : (j + ng) * w * 8],
                        )
                        ldst_gtile = meta_pool.tile([P, G * msc], BF16, tag="ldst")
                        nc.sync.dma_start(
                            ldst_gtile[:, : ng * msc],
                            ldst_t[:, j * msc : (j + ng) * msc],
                        )
                    jg = j % G
                    msgs = gather_pool.tile([P, w, elem], BF16)
                    nc.gpsimd.dma_gather(
                        out_ap=msgs[:, :, :],
                        in_ap=x_t[b * block : min((b + 1) * block, n_nodes), :],
                        idxs_ap=idx_gtile[:, jg * w * 8 : (jg + 1) * w * 8],
                        num_idxs=w * P,
                        num_idxs_reg=nidx_reg,
                        elem_size=elem,
                        single_packet=SINGLE_PACKET,
                        queue_num=queue,
                    )
                    for j0 in range(0, nsc, 8):
                        g = min(8, nsc - j0)
                        onehot = oh_pool.tile([P, 8 * P], BF16, name="oh", tag="oh")
                        lt = ldst_gtile[:, jg * msc + j0 : jg * msc + j0 + g]
                        lt_b = bass.AP(lt.tensor, lt.offset, lt.ap + [[0, P]])
                        nc.vector.tensor_tensor(
                            out=onehot[:, : g * P].rearrange(
                                "p (g q) -> p g q", q=P
                            ),
                            in0=iota_b[:, : g * P].rearrange(
                                "p (g q) -> p g q", q=P
                            ),
                            in1=lt_b,
                            op=mybir.AluOpType.is_equal,
                        )
                        for jj in range(g):
                            si = s0 + j0 + jj
                            t = int(mm_tile[si])
                            cin = int(mm_chunk[si]) - c0
                            if mm_first[si]:
                                psums[t] = psum_pool.tile(
                                    [P, elem], F32, tag="ps", name=f"ps{t}"
                                )
                            nc.tensor.matmul(
                                psums[t][:, :],
                                lhsT=onehot[:, jj * P : (jj + 1) * P],
                                rhs=msgs[:, cin, :],
                                start=bool(mm_first[si]),
                                stop=bool(mm_last[si]),
                            )
                # evacuate: stage[:, t*dim:+dim] = psum_hi + psum_lo
                for t in ts:
                    if not tile_has[t]:
                        continue
                    ps = psums.pop(t)
                    nc.scalar.copy(stage[:, t * dim : (t + 1) * dim], ps[:, :dim])
                    nc.vector.tensor_tensor(
                        out=stage[:, t * dim : (t + 1) * dim],
                        in0=stage[:, t * dim : (t + 1) * dim],
                        in1=ps[:, dim:],
                        op=mybir.AluOpType.add,
                    )

            out_view = out_t[:, :].rearrange("(t p) d -> p t d", p=P)
            nc.sync.dma_start(out_view, stage[:])

    nc.compile()
    return nc


def _run(x, edge_index, n_nodes, dim, n_cores, block, w, **run_kwargs):
    sched = _prep(edge_index, n_nodes, n_cores, block, w)
    xp = _pack_x(x)
    nc = _build(n_nodes, dim, block, w, sched)
    in_maps = [
        {"xpack": xp, "idx": sched["idx"][k], "ldst": sched["ldst"][k]}
        for k in range(n_cores)
    ]
    res = run_bass_kernel_spmd(
        nc, in_maps, core_ids=list(range(n_cores)), **run_kwargs
    )
    npc = sched["npc"]
    tiles = sched["tiles"]
    perm = sched["perm"]
    parts = []
    for k in range(n_cores):
        r = res.results[k]["out"].reshape(tiles, P, -1)
        inv = np.empty(tiles, np.int64)
        inv[perm[k]] = np.arange(tiles)
        parts.append(r[inv].reshape(tiles * P, -1)[:npc])
    out = np.concatenate(parts, axis=0)
    return out, res


def kernel(x, edge_index):
    out, _ = _run(
        x, edge_index, N_NODES, DIM, N_CORES, SRC_BLOCK, CHUNKS_PER_CALL
    )
    return out
